# revision 9
# baseline (speedup 1.0000x reference)
"""MDCT kernel for Trainium2 (8 NeuronCores, batch-parallel), folded DCT-IV form.

Math: frame f (hop N=1024, len 2N, center-padded) folds to an N-vector u and
out[f] = DCT-IV(u).  With x2 = x.reshape(1024, 1024) and y1 = w[:N]*x2[r],
y2 = w[N:]*x2[r] (per-row windowing):
    u[f, m]      = -y2[f, 511-m] - y2[f, 512+m]      (m < 512,  row f)
    u[f, 512+p]  =  y1[f-1, p]   - y1[f-1, 1023-p]   (p < 512,  row f-1)
so each x2 row r yields uLo[r] (frame r) and uHi[r] (frame r+1), and
    out[f, k] = sum_m u[f, m] * D4[m, k],   D4 = sqrt(2/N) DCT-IV matrix.

This halves the matmul contraction (1024 vs 2048) vs the direct form.  The
fold runs on the vector engine (reversals are negative-stride APs), u is
transposed on the PE in bf16 (1 cyc/row), and the DCT matmuls run in bf16.

Only the LEFT half of D4 is DMA'd (1.05 MB instead of 2.1 MB): the DCT-IV
matrix satisfies  D4[m, 512+k] = s_m*sqrt(2)*D4[m, k] - D4[m, 511-k]  with
s_m = +1 for m%4 in {0,3} else -1, so the right half is derived on the
(otherwise idle) GpSimd engine with one per-partition-scalar multiply and
one reversed-AP subtract per 128-row chunk.

Schedule notes (v3, from NTFF profiles of the 53.8us baseline and a 66.8us
batched-DMA attempt):
- the NEFF preamble (engine iram loads, const memsets, a global barrier)
  ends ~7.2 us; nothing (including DMA) starts before that.
- per-DMA-instruction bandwidth is limited; aggregate ~400 GB/s needs
  several instructions in flight, so fills stay as ~0.25-0.5 MB pieces.
- every dma_start costs ~0.6-0.7 us of issue time on its engine's queue;
  fills go on Sync, output stores are split per half-tile and issued by
  the engine that produced that half (ACT for k<512, GpSimd for k>=512),
  so stores never wait on cross-engine semaphores or queue behind fills.
- fill order: sv, w1, x0, w2n, Dl45, Dl67, x1, Dl01, Dl23, x2..x7 - the
  DCT gate (all left-D chunks + derived rights) clears ~12.5 us and x1
  lands in time for fold(1) to feed dct_tile(1) with no PE gap.
- warmup transposes bridge the HAM clock gate (ramps to 2.4 GHz after
  ~3.5 us of sustained PE activity; >3.4 us idle re-throttles).
- engine ownership (GPSIMD cannot touch PSUM): DVE = folds + uLo
  staging + pb copies; ACT = uHi staging + pa copies + both half-tile
  stores; GpSimd = right-D derivation; PE = warmup, fold transposes,
  DCT chains, last frame.
- frame 1024 (uHi of row 1023 only) runs as a 1-partition 8-matmul chain
  after dct_tile(7); engine APs cannot start at partition 1 (HW is
  32-granular) so it cannot be folded into a shifted-psum combine.
"""

import numpy as np
import ml_dtypes

import concourse.bass as bass
import concourse.bacc as bacc
import concourse.mybir as mybir
import concourse.tile as tile
from concourse import masks
from concourse.bass_utils import run_bass_kernel_spmd

B = 8
T = 1 << 20
R = 1024          # rows of x2 per channel (T // hop)
CN = 1024         # row width (hop) = N
NF = 1025         # output frames
NK = 1024         # output bins
F32 = mybir.dt.float32
BF16 = mybir.dt.bfloat16

_NC_CACHE = None
_CONST_CACHE = None


def build_nc() -> bass.Bass:
    nc = bacc.Bacc("TRN2", target_bir_lowering=False, debug=False)
    x = nc.dram_tensor("x", [R, CN], BF16, kind="ExternalInput").ap()
    w1r = nc.dram_tensor("w1r", [128, CN], BF16, kind="ExternalInput").ap()
    w2nr = nc.dram_tensor("w2nr", [128, CN], BF16, kind="ExternalInput").ap()
    d4l = nc.dram_tensor("d4l", [8, 128, 512], BF16, kind="ExternalInput").ap()
    svr = nc.dram_tensor("svr", [128, 1], F32, kind="ExternalInput").ap()
    out = nc.dram_tensor("out", [NF, NK], BF16, kind="ExternalOutput").ap()

    xv = x.rearrange("(a p) c -> p a c", p=128)
    dv = d4l.rearrange("a p c -> p a c")

    with tile.TileContext(nc) as tc:
        with (
            tc.tile_pool(name="persist", bufs=1) as persist,
            tc.tile_pool(name="xin", bufs=1) as xin,
            tc.tile_pool(name="ypool", bufs=6) as ypool,
            tc.tile_pool(name="upool", bufs=4) as upool,
            tc.tile_pool(name="dtmp", bufs=2) as dtmp,
            tc.tile_pool(name="outp", bufs=4) as outp,
            tc.tile_pool(name="wps", bufs=1, space="PSUM") as wps,
            tc.tile_pool(name="tps", bufs=2, space="PSUM") as tps,
            tc.tile_pool(name="mmps", bufs=4, space="PSUM") as mmps,
        ):
            w1 = persist.tile([128, CN], BF16)
            w2n = persist.tile([128, CN], BF16)
            sv = persist.tile([128, 1], F32)

            ident = persist.tile([128, 128], BF16)
            masks.make_identity(nc, ident[:])

            dt = persist.tile([128, 8, NK], BF16)
            ulot = persist.tile([128, 4, R], BF16)
            uhit = persist.tile([128, 4, NF], BF16)
            nc.vector.memset(uhit[:, :, 0:1], 0.0)

            xts = [xin.tile([128, CN], BF16, name=f"xt{i}") for i in range(8)]

            # PE warmup: keep the HAM clock gate fed until fold(0)'s
            # transposes arrive, so 2.4 GHz is reached before the DCT.
            warm = wps.tile([128, 128], BF16, tag="warm")
            for _ in range(20):
                nc.tensor.transpose(warm[:], ident[:], ident[:])

            # Fill DMAs (Sync queue), critical-path first.
            nc.sync.dma_start(sv[:], svr)
            nc.sync.dma_start(w1[:], w1r)
            nc.sync.dma_start(xts[0][:], xv[:, 0, :])
            nc.sync.dma_start(w2n[:], w2nr)
            nc.sync.dma_start(dt[:, 4:6, 0:512], dv[:, 4:6, :])
            nc.sync.dma_start(dt[:, 6:8, 0:512], dv[:, 6:8, :])
            nc.sync.dma_start(xts[1][:], xv[:, 1, :])
            nc.sync.dma_start(dt[:, 0:2, 0:512], dv[:, 0:2, :])
            nc.sync.dma_start(dt[:, 2:4, 0:512], dv[:, 2:4, :])
            for r in range(2, 8):
                nc.sync.dma_start(xts[r][:], xv[:, r, :])

            # Derive right half of D on GpSimd:
            #   dt[:, ci, 512+k] = sv*dt[:, ci, k] - dt[:, ci, 511-k]
            def derive(ci):
                tmp = dtmp.tile([128, 512], BF16)
                nc.gpsimd.tensor_scalar_mul(tmp[:], dt[:, ci, 0:512], sv[:, 0:1])
                nc.gpsimd.tensor_tensor(
                    dt[:, ci, 512:1024], tmp[:], dt[:, ci, 511::-1],
                    mybir.AluOpType.subtract,
                )

            for ci in (4, 5, 6, 7, 0, 1, 2, 3):
                derive(ci)

            def fold(r: int):
                xt = xts[r][:]
                r0 = r * 128
                pt = tps.tile([128, CN], BF16, tag="pt")
                y1 = ypool.tile([128, CN], BF16, tag="y1")
                un = upool.tile([128, CN], BF16)
                nc.vector.tensor_tensor(y1[:], xt, w1[:], mybir.AluOpType.mult)
                # uHi[p] = y1[p] - y1[1023-p]
                nc.vector.tensor_tensor(
                    un[:, 512:1024], y1[:, 0:512], y1[:, 1023:511:-1],
                    mybir.AluOpType.subtract,
                )
                for ci in range(4):
                    nc.tensor.transpose(
                        pt[:, ci * 128:(ci + 1) * 128],
                        un[:, 512 + ci * 128:512 + (ci + 1) * 128], ident[:],
                    )
                nc.scalar.copy(uhit[:, 0:4, 1 + r0:1 + r0 + 128], pt[:, 0:512])
                y2n = ypool.tile([128, CN], BF16, tag="y2n")
                nc.vector.tensor_tensor(y2n[:], xt, w2n[:], mybir.AluOpType.mult)
                # uLo[m] = y2n[511-m] + y2n[512+m]   (y2n = -w2*x)
                nc.vector.tensor_tensor(
                    un[:, 0:512], y2n[:, 511::-1], y2n[:, 512:1024],
                    mybir.AluOpType.add,
                )
                for ci in range(4):
                    nc.tensor.transpose(
                        pt[:, 512 + ci * 128:512 + (ci + 1) * 128],
                        un[:, ci * 128:(ci + 1) * 128], ident[:],
                    )
                nc.vector.tensor_copy(ulot[:, 0:4, r0:r0 + 128], pt[:, 512:1024])

            def wslice(ci, f0):
                if ci < 4:
                    return ulot[:, ci, f0:f0 + 128]
                return uhit[:, ci - 4, f0:f0 + 128]

            CHAIN = (4, 5, 6, 7, 0, 1, 2, 3)

            def dct_tile(j: int):
                f0 = j * 128
                ot = outp.tile([128, NK], BF16)
                pa = mmps.tile([128, 512], F32, tag="mm")
                for ci in CHAIN:
                    nc.tensor.matmul(
                        pa[:], wslice(ci, f0), dt[:, ci, 0:512],
                        start=(ci == CHAIN[0]), stop=(ci == CHAIN[-1]),
                    )
                nc.scalar.copy(ot[:, 0:512], pa[:])
                nc.scalar.dma_start(out[f0:f0 + 128, 0:512], ot[:, 0:512])
                pb = mmps.tile([128, 512], F32, tag="mm")
                for ci in CHAIN:
                    nc.tensor.matmul(
                        pb[:], wslice(ci, f0), dt[:, ci, 512:1024],
                        start=(ci == CHAIN[0]), stop=(ci == CHAIN[-1]),
                    )
                nc.vector.tensor_copy(ot[:, 512:1024], pb[:])
                nc.scalar.dma_start(out[f0:f0 + 128, 512:1024], ot[:, 512:1024])

            def last_frame():
                # f=1024: only the uHi half (row 1023) contributes.
                pa = mmps.tile([1, 512], F32, tag="mm")
                pb = mmps.tile([1, 512], F32, tag="mm")
                for ci in range(4):
                    wsl = uhit[:, ci, 1024:1025]
                    nc.tensor.matmul(
                        pa[:], wsl, dt[:, 4 + ci, 0:512],
                        start=(ci == 0), stop=(ci == 3),
                    )
                    nc.tensor.matmul(
                        pb[:], wsl, dt[:, 4 + ci, 512:1024],
                        start=(ci == 0), stop=(ci == 3),
                    )
                ot = outp.tile([1, NK], BF16, tag="ot_last")
                nc.scalar.copy(ot[:, 0:512], pa[:])
                nc.scalar.dma_start(out[1024:1025, 0:512], ot[:, 0:512])
                nc.vector.tensor_copy(ot[:, 512:1024], pb[:])
                nc.scalar.dma_start(out[1024:1025, 512:1024], ot[:, 512:1024])

            fold(0)
            dct_tile(0)
            for r in range(1, 8):
                fold(r)
                dct_tile(r)
            last_frame()

    return nc


def make_consts(window: np.ndarray):
    w = window.astype(np.float64)
    w1r = np.broadcast_to(w[:CN].astype(ml_dtypes.bfloat16), (128, CN)).copy()
    w2nr = np.broadcast_to((-w[CN:]).astype(ml_dtypes.bfloat16), (128, CN)).copy()
    m = np.arange(NK, dtype=np.float64)[:, None]
    k = np.arange(NK, dtype=np.float64)[None, :]
    d = (np.sqrt(2.0 / NK) * np.cos(np.pi / NK * (m + 0.5) * (k + 0.5)))
    d4l = np.ascontiguousarray(
        d.astype(ml_dtypes.bfloat16).reshape(8, 128, NK)[:, :, :512])
    p = np.arange(128)
    svr = np.where(np.isin(p % 4, [0, 3]), np.sqrt(2.0), -np.sqrt(2.0))
    svr = svr.reshape(128, 1).astype(np.float32)
    return w1r, w2nr, d4l, svr


def _get_nc() -> bass.Bass:
    global _NC_CACHE
    if _NC_CACHE is None:
        _NC_CACHE = build_nc()
        _NC_CACHE.compile()
    return _NC_CACHE


def run_spmd(x: np.ndarray, window: np.ndarray, **kwargs):
    """Shard, run on 8 cores, return (stacked output, BassKernelResults)."""
    global _CONST_CACHE
    if _CONST_CACHE is None or _CONST_CACHE[0] != window.tobytes():
        _CONST_CACHE = (window.tobytes(), make_consts(window))
    w1r, w2nr, d4l, svr = _CONST_CACHE[1]
    in_maps = [
        {"x": np.ascontiguousarray(
            x[b].reshape(R, CN).astype(ml_dtypes.bfloat16)),
         "w1r": w1r, "w2nr": w2nr, "d4l": d4l, "svr": svr}
        for b in range(B)
    ]
    res = run_bass_kernel_spmd(nc=_get_nc(), in_maps=in_maps,
                               core_ids=list(range(B)), **kwargs)
    out = np.stack([res.results[b]["out"].astype(np.float32) for b in range(B)],
                   axis=0)
    return out, res


def kernel(x: np.ndarray, window: np.ndarray) -> np.ndarray:
    out, _ = run_spmd(np.asarray(x), np.asarray(window))
    return out


# revision 10
# speedup vs baseline: 2.1585x; 2.1585x over previous
"""MDCT kernel for Trainium2 (8 NeuronCores, batch-parallel), folded DCT-IV form.

Math: frame f (hop N=1024, len 2N, center-padded) folds to an N-vector u and
out[f] = DCT-IV(u).  With x2 = x.reshape(1024, 1024) and y1 = w[:N]*x2[r],
y2 = w[N:]*x2[r] (per-row windowing):
    u[f, m]      = -y2[f, 511-m] - y2[f, 512+m]      (m < 512,  row f)
    u[f, 512+p]  =  y1[f-1, p]   - y1[f-1, 1023-p]   (p < 512,  row f-1)
so each x2 row r yields uLo[r] (frame r) and uHi[r] (frame r+1), and
    out[f, k] = sum_m u[f, m] * D4[m, k],   D4 = sqrt(2/N) DCT-IV matrix.

This halves the matmul contraction (1024 vs 2048) vs the direct form.  The
fold runs on the vector engine (reversals are negative-stride APs), u is
transposed on the PE in bf16 (1 cyc/row), and the DCT matmuls run in bf16.

(The DCT-IV right half satisfies D4[m,512+k] = s_m*sqrt(2)*D4[m,k] -
D4[m,511-k], but deriving it on-chip is a loss: GPSIMD runs tensor_scalar
at ~7.6us per [128,512] chunk and DVE/ACT have no spare capacity early,
so the full 2.1 MB D4 is DMA'd.)

Schedule notes (v3, from NTFF profiles of the 53.8us baseline and a 66.8us
batched-DMA attempt):
- the NEFF preamble (engine iram loads, const memsets, a global barrier)
  ends ~7.2 us; nothing (including DMA) starts before that.
- per-DMA-instruction bandwidth is limited; aggregate ~400 GB/s needs
  several instructions in flight, so fills stay as ~0.25-0.5 MB pieces.
- every dma_start costs ~0.6-0.7 us of issue time on its engine's queue;
  fills go on Sync, output stores are split per half-tile and issued by
  the engine that produced that half (ACT for k<512, GpSimd for k>=512),
  so stores never wait on cross-engine semaphores or queue behind fills.
- fill order: w1, x0, w2n, D45, D67, x1, D01, D23, x2..x7 - the DCT
  gate (all of D) clears ~13 us and x1 lands in time for fold(1) to
  feed dct_tile(1) with no PE gap.
- warmup transposes bridge the HAM clock gate (ramps to 2.4 GHz after
  ~3.5 us of sustained PE activity; >3.4 us idle re-throttles).
- engine ownership (GPSIMD cannot touch PSUM, and is ~25x slower than
  DVE on element-wise ops): DVE = folds + uLo staging + pb copies;
  ACT = uHi staging + pa copies + both half-tile stores; PE = warmup,
  fold transposes, DCT chains, last frame.
- frame 1024 (uHi of row 1023 only) runs as a 1-partition 8-matmul chain
  after dct_tile(7); engine APs cannot start at partition 1 (HW is
  32-granular) so it cannot be folded into a shifted-psum combine.
"""

import numpy as np
import ml_dtypes

import concourse.bass as bass
import concourse.bacc as bacc
import concourse.mybir as mybir
import concourse.tile as tile
from concourse import masks
from concourse.bass_utils import run_bass_kernel_spmd

B = 8
T = 1 << 20
R = 1024          # rows of x2 per channel (T // hop)
CN = 1024         # row width (hop) = N
NF = 1025         # output frames
NK = 1024         # output bins
F32 = mybir.dt.float32
BF16 = mybir.dt.bfloat16

_NC_CACHE = None
_CONST_CACHE = None


def build_nc() -> bass.Bass:
    nc = bacc.Bacc("TRN2", target_bir_lowering=False, debug=False)
    x = nc.dram_tensor("x", [R, CN], BF16, kind="ExternalInput").ap()
    w1r = nc.dram_tensor("w1r", [128, CN], BF16, kind="ExternalInput").ap()
    w2nr = nc.dram_tensor("w2nr", [128, CN], BF16, kind="ExternalInput").ap()
    d4 = nc.dram_tensor("d4", [8, 128, NK], BF16, kind="ExternalInput").ap()
    out = nc.dram_tensor("out", [NF, NK], BF16, kind="ExternalOutput").ap()

    xv = x.rearrange("(a p) c -> p a c", p=128)
    dv = d4.rearrange("a p c -> p a c")

    with tile.TileContext(nc) as tc:
        with (
            tc.tile_pool(name="persist", bufs=1) as persist,
            tc.tile_pool(name="xin", bufs=1) as xin,
            tc.tile_pool(name="ypool", bufs=6) as ypool,
            tc.tile_pool(name="upool", bufs=4) as upool,
            tc.tile_pool(name="outp", bufs=4) as outp,
            tc.tile_pool(name="wps", bufs=1, space="PSUM") as wps,
            tc.tile_pool(name="tps", bufs=2, space="PSUM") as tps,
            tc.tile_pool(name="mmps", bufs=4, space="PSUM") as mmps,
        ):
            w1 = persist.tile([128, CN], BF16)
            w2n = persist.tile([128, CN], BF16)

            ident = persist.tile([128, 128], BF16)
            masks.make_identity(nc, ident[:])

            dt = persist.tile([128, 8, NK], BF16)
            ulot = persist.tile([128, 4, R], BF16)
            uhit = persist.tile([128, 4, NF], BF16)
            nc.vector.memset(uhit[:, :, 0:1], 0.0)

            xts = [xin.tile([128, CN], BF16, name=f"xt{i}") for i in range(8)]

            # PE warmup: keep the HAM clock gate fed until fold(0)'s
            # transposes arrive, so 2.4 GHz is reached before the DCT.
            warm = wps.tile([128, 128], BF16, tag="warm")
            for _ in range(20):
                nc.tensor.transpose(warm[:], ident[:], ident[:])

            # Fill DMAs (Sync queue), critical-path first.
            nc.sync.dma_start(w1[:], w1r)
            nc.sync.dma_start(xts[0][:], xv[:, 0, :])
            nc.sync.dma_start(w2n[:], w2nr)
            nc.sync.dma_start(dt[:, 4:6, :], dv[:, 4:6, :])
            nc.sync.dma_start(dt[:, 6:8, :], dv[:, 6:8, :])
            nc.sync.dma_start(xts[1][:], xv[:, 1, :])
            nc.sync.dma_start(dt[:, 0:2, :], dv[:, 0:2, :])
            nc.sync.dma_start(dt[:, 2:4, :], dv[:, 2:4, :])
            for r in range(2, 8):
                nc.sync.dma_start(xts[r][:], xv[:, r, :])

            def fold(r: int):
                xt = xts[r][:]
                r0 = r * 128
                pt = tps.tile([128, CN], BF16, tag="pt")
                y1 = ypool.tile([128, CN], BF16, tag="y1")
                un = upool.tile([128, CN], BF16)
                nc.vector.tensor_tensor(y1[:], xt, w1[:], mybir.AluOpType.mult)
                # uHi[p] = y1[p] - y1[1023-p]
                nc.vector.tensor_tensor(
                    un[:, 512:1024], y1[:, 0:512], y1[:, 1023:511:-1],
                    mybir.AluOpType.subtract,
                )
                for ci in range(4):
                    nc.tensor.transpose(
                        pt[:, ci * 128:(ci + 1) * 128],
                        un[:, 512 + ci * 128:512 + (ci + 1) * 128], ident[:],
                    )
                nc.scalar.copy(uhit[:, 0:4, 1 + r0:1 + r0 + 128], pt[:, 0:512])
                y2n = ypool.tile([128, CN], BF16, tag="y2n")
                nc.vector.tensor_tensor(y2n[:], xt, w2n[:], mybir.AluOpType.mult)
                # uLo[m] = y2n[511-m] + y2n[512+m]   (y2n = -w2*x)
                nc.vector.tensor_tensor(
                    un[:, 0:512], y2n[:, 511::-1], y2n[:, 512:1024],
                    mybir.AluOpType.add,
                )
                for ci in range(4):
                    nc.tensor.transpose(
                        pt[:, 512 + ci * 128:512 + (ci + 1) * 128],
                        un[:, ci * 128:(ci + 1) * 128], ident[:],
                    )
                nc.vector.tensor_copy(ulot[:, 0:4, r0:r0 + 128], pt[:, 512:1024])

            def wslice(ci, f0):
                if ci < 4:
                    return ulot[:, ci, f0:f0 + 128]
                return uhit[:, ci - 4, f0:f0 + 128]

            CHAIN = (4, 5, 6, 7, 0, 1, 2, 3)

            def dct_tile(j: int):
                f0 = j * 128
                ot = outp.tile([128, NK], BF16)
                pa = mmps.tile([128, 512], F32, tag="mm")
                for ci in CHAIN:
                    nc.tensor.matmul(
                        pa[:], wslice(ci, f0), dt[:, ci, 0:512],
                        start=(ci == CHAIN[0]), stop=(ci == CHAIN[-1]),
                    )
                nc.scalar.copy(ot[:, 0:512], pa[:])
                nc.scalar.dma_start(out[f0:f0 + 128, 0:512], ot[:, 0:512])
                pb = mmps.tile([128, 512], F32, tag="mm")
                for ci in CHAIN:
                    nc.tensor.matmul(
                        pb[:], wslice(ci, f0), dt[:, ci, 512:1024],
                        start=(ci == CHAIN[0]), stop=(ci == CHAIN[-1]),
                    )
                nc.vector.tensor_copy(ot[:, 512:1024], pb[:])
                nc.scalar.dma_start(out[f0:f0 + 128, 512:1024], ot[:, 512:1024])

            def last_frame():
                # f=1024: only the uHi half (row 1023) contributes.
                pa = mmps.tile([1, 512], F32, tag="mm")
                pb = mmps.tile([1, 512], F32, tag="mm")
                for ci in range(4):
                    wsl = uhit[:, ci, 1024:1025]
                    nc.tensor.matmul(
                        pa[:], wsl, dt[:, 4 + ci, 0:512],
                        start=(ci == 0), stop=(ci == 3),
                    )
                    nc.tensor.matmul(
                        pb[:], wsl, dt[:, 4 + ci, 512:1024],
                        start=(ci == 0), stop=(ci == 3),
                    )
                ot = outp.tile([1, NK], BF16, tag="ot_last")
                nc.scalar.copy(ot[:, 0:512], pa[:])
                nc.scalar.dma_start(out[1024:1025, 0:512], ot[:, 0:512])
                nc.vector.tensor_copy(ot[:, 512:1024], pb[:])
                nc.scalar.dma_start(out[1024:1025, 512:1024], ot[:, 512:1024])

            fold(0)
            dct_tile(0)
            for r in range(1, 8):
                fold(r)
                dct_tile(r)
            last_frame()

    return nc


def make_consts(window: np.ndarray):
    w = window.astype(np.float64)
    w1r = np.broadcast_to(w[:CN].astype(ml_dtypes.bfloat16), (128, CN)).copy()
    w2nr = np.broadcast_to((-w[CN:]).astype(ml_dtypes.bfloat16), (128, CN)).copy()
    m = np.arange(NK, dtype=np.float64)[:, None]
    k = np.arange(NK, dtype=np.float64)[None, :]
    d = (np.sqrt(2.0 / NK) * np.cos(np.pi / NK * (m + 0.5) * (k + 0.5)))
    d4 = d.astype(ml_dtypes.bfloat16).reshape(8, 128, NK)
    return w1r, w2nr, d4


def _get_nc() -> bass.Bass:
    global _NC_CACHE
    if _NC_CACHE is None:
        _NC_CACHE = build_nc()
        _NC_CACHE.compile()
    return _NC_CACHE


def run_spmd(x: np.ndarray, window: np.ndarray, **kwargs):
    """Shard, run on 8 cores, return (stacked output, BassKernelResults)."""
    global _CONST_CACHE
    if _CONST_CACHE is None or _CONST_CACHE[0] != window.tobytes():
        _CONST_CACHE = (window.tobytes(), make_consts(window))
    w1r, w2nr, d4 = _CONST_CACHE[1]
    in_maps = [
        {"x": np.ascontiguousarray(
            x[b].reshape(R, CN).astype(ml_dtypes.bfloat16)),
         "w1r": w1r, "w2nr": w2nr, "d4": d4}
        for b in range(B)
    ]
    res = run_bass_kernel_spmd(nc=_get_nc(), in_maps=in_maps,
                               core_ids=list(range(B)), **kwargs)
    out = np.stack([res.results[b]["out"].astype(np.float32) for b in range(B)],
                   axis=0)
    return out, res


def kernel(x: np.ndarray, window: np.ndarray) -> np.ndarray:
    out, _ = run_spmd(np.asarray(x), np.asarray(window))
    return out
